# revision 9
# baseline (speedup 1.0000x reference)
"""Multi-head attention forward on 8 TRN2 NeuronCores.

Data-parallel over batch: B=8 batch elements -> one per core, zero
collectives. Per core (one batch element, N=1024 tokens, D=768, H=12
heads of HD=64):

  1. qkT = (x @ w_qkv[:, :2D]).T + b      transposed layout [2D, N]
  2. v   =  x @ w_qkv[:, 2D:]  + b        natural layout [N, D], stored
                                          per-head with a ones column
  3. per head: scoresT[k, q] = k_h q_h^T  (K=64 matmuls)
     expT = exp(SCALE * scoresT)          bf16, no max subtraction
                                          (scores ~ N(0,1), exp safe)
  4. av[q, 0:64] = sum_k expT * v_h ; av[q, 64] = sum_k expT  (ones col
     gives the softmax denominator for free); normalize per-partition
  5. attn_out -> PE transpose -> proj matmul + bias -> out

Everything computes in bf16 with fp32 PSUM accumulation (fp8 was
measured at 3.5-5% output error on every path -- the 2e-2 gate rules
it out; the weighted-average structure of attention does not attenuate
v/p quantization noise because the output is itself noise-sized).

Emission is interleaved per head-pair so ACT (the exp engine, ~110us
total) starts early and overlaps the whole PE timeline.

Host-side prep (outside the HW kernel): shard x over batch, transpose
x to [D, N], cast x/w to bf16.
"""

from contextlib import ExitStack

import numpy as np
import ml_dtypes

import concourse.bass as bass
import concourse.mybir as mybir
import concourse.tile as tile
from concourse import bacc
from concourse.masks import make_identity
from concourse.bass_utils import run_bass_kernel_spmd

B, N, D, H = 8, 1024, 768, 12
HD = D // H  # 64
SCALE = HD ** -0.5
BF16 = mybir.dt.bfloat16
F32 = mybir.dt.float32
BCOLS = HD + 1  # v block width per head: 64 v columns + ones column

_CACHED_NC = {}


def build_graph(n=N):
    """Build the single-core Bass graph (SPMD: same graph on all 8 cores)."""
    assert n % 128 == 0
    DP = D // 128           # 6 chunks of the model dim
    NQ = n // 128           # seq chunks
    NP = 2 * D // 128 // 2  # 6 head-pair chunks (q or k each)
    qwins = [(s, min(512, n - s)) for s in range(0, n, 512)]
    pwins = [(s, min(384, D - s)) for s in range(0, D, 384)]

    nc = bacc.Bacc()
    xt = nc.declare_dram_parameter("xt", [D, n], BF16, isOutput=False)
    wqkv = nc.declare_dram_parameter("w_qkv", [D, 3 * D], BF16, isOutput=False)
    bqkv = nc.declare_dram_parameter("b_qkv", [3 * D], F32, isOutput=False)
    wproj = nc.declare_dram_parameter("w_proj", [D, D], BF16, isOutput=False)
    bproj = nc.declare_dram_parameter("b_proj", [D], F32, isOutput=False)
    out = nc.declare_dram_parameter("out", [n, D], F32, isOutput=True)

    xt_v = xt[:].rearrange("(dc p) n -> p dc n", p=128)
    wqkv_v = wqkv[:].rearrange("(dc p) m -> p dc m", p=128)
    wproj_v = wproj[:].rearrange("(dc p) m -> p dc m", p=128)
    out_v = out[:].rearrange("(c p) m -> p c m", p=128)
    bqk_v = bqkv[: 2 * D].rearrange("(j p) -> p j", p=128)

    def bcast(ap, p=128):
        return bass.AP(tensor=ap.tensor, offset=ap.offset, ap=[[0, p]] + list(ap.ap))

    with ExitStack() as ctx:
        tc = ctx.enter_context(tile.TileContext(nc))
        const = ctx.enter_context(tc.tile_pool(name="const", bufs=1))
        persist = ctx.enter_context(tc.tile_pool(name="persist", bufs=1))
        exps = ctx.enter_context(tc.tile_pool(name="exps", bufs=2))
        smalls = ctx.enter_context(tc.tile_pool(name="smalls", bufs=4))
        ps_mm = ctx.enter_context(tc.tile_pool(name="ps_mm", bufs=2, space="PSUM"))
        ps_sc = ctx.enter_context(tc.tile_pool(name="ps_sc", bufs=2, space="PSUM"))
        ps_av = ctx.enter_context(tc.tile_pool(name="ps_av", bufs=2, space="PSUM"))

        # ---- loads ----
        xt_sb = persist.tile([128, DP, n], BF16, tag="xt")
        for dc in range(DP):
            nc.sync.dma_start(out=xt_sb[:, dc, :], in_=xt_v[:, dc, :])
        wq_sb = persist.tile([128, DP, 3 * D], BF16, tag="wq")
        for dc in range(DP):
            nc.sync.dma_start(out=wq_sb[:, dc, :], in_=wqkv_v[:, dc, :])
        # biases routed through one DVE copy so later DVE adds depend on
        # them via same-engine program order (walrus allows 1 wait/inst)
        bqk_ld = const.tile([128, 2 * D // 128], F32, tag="bqk_ld")
        nc.gpsimd.dma_start(out=bqk_ld[:], in_=bqk_v)
        bqk_sb = const.tile([128, 2 * D // 128], F32, tag="bqk")
        nc.vector.tensor_copy(out=bqk_sb[:], in_=bqk_ld[:])
        bv_ld = const.tile([128, D], F32, tag="bv_ld")
        nc.gpsimd.dma_start(out=bv_ld[:], in_=bcast(bqkv[2 * D : 3 * D]))
        bv_sb = const.tile([128, D], F32, tag="bv")
        nc.vector.tensor_copy(out=bv_sb[:], in_=bv_ld[:])
        bp_ld = const.tile([128, D], F32, tag="bp_ld")
        nc.gpsimd.dma_start(out=bp_ld[:], in_=bcast(bproj[:]))
        bp_sb = const.tile([128, D], F32, tag="bp")
        nc.vector.tensor_copy(out=bp_sb[:], in_=bp_ld[:])
        wp_sb = persist.tile([128, DP, D], BF16, tag="wp")
        for dc in range(DP):
            nc.sync.dma_start(out=wp_sb[:, dc, :], in_=wproj_v[:, dc, :])
        ident = const.tile([128, 128], BF16, tag="ident")
        make_identity(nc, ident)

        qkT_sb = persist.tile([128, 2 * D // 128, n], BF16, tag="qkT")
        v_sb = persist.tile([128, NQ, H, BCOLS], BF16, tag="v")
        nc.vector.memset(v_sb[:, :, :, HD : HD + 1], 1.0)
        attn_sb = persist.tile([128, NQ, D], BF16, tag="attn")
        attnT_sb = persist.tile([128, DP, n], BF16, tag="attnT")
        et_tiles = [None] * H

        def emit_qkT(j):
            # one 128-wide chunk of the transposed q|k projection
            for s, L in qwins:
                pt = ps_mm.tile([128, 512], F32, tag="mm")
                for dc in range(DP):
                    nc.tensor.matmul(
                        pt[:, :L],
                        lhsT=wq_sb[:, dc, j * 128 : (j + 1) * 128],
                        rhs=xt_sb[:, dc, s : s + L],
                        start=(dc == 0),
                        stop=(dc == DP - 1),
                    )
                nc.vector.tensor_scalar_add(
                    out=qkT_sb[:, j, s : s + L],
                    in0=pt[:, :L],
                    scalar1=bqk_sb[:, j : j + 1],
                )

        def emit_v():
            for c in range(NQ):
                for s, L in pwins:
                    nh = L // HD
                    h0 = s // HD
                    pt = ps_mm.tile([128, 512], F32, tag="mm")
                    for dc in range(DP):
                        nc.tensor.matmul(
                            pt[:, :L],
                            lhsT=xt_sb[:, dc, c * 128 : (c + 1) * 128],
                            rhs=wq_sb[:, dc, 2 * D + s : 2 * D + s + L],
                            start=(dc == 0),
                            stop=(dc == DP - 1),
                        )
                    nc.vector.tensor_add(
                        v_sb[:, c, h0 : h0 + nh, 0:HD],
                        pt[:, :L].rearrange("p (h x) -> p h x", h=nh),
                        bv_sb[:, s : s + L].rearrange("p (h x) -> p h x", h=nh),
                    )

        def emit_scores(h):
            poff = (h % 2) * 64
            qch = h // 2
            kch = NP + h // 2
            et = exps.tile([128, NQ, n], BF16, tag="exp")
            et_tiles[h] = et
            for kc in range(NQ):
                st = ps_sc.tile([128, n], F32, tag="sc")
                for s, L in qwins:
                    nc.tensor.matmul(
                        st[:, s : s + L],
                        lhsT=qkT_sb[poff : poff + 64, kch, kc * 128 : (kc + 1) * 128],
                        rhs=qkT_sb[poff : poff + 64, qch, s : s + L],
                        start=True,
                        stop=True,
                    )
                nc.scalar.activation(
                    out=et[:, kc, :],
                    in_=st[:, :],
                    func=mybir.ActivationFunctionType.Exp,
                    scale=SCALE,
                )

        def emit_av(h):
            et = et_tiles[h]
            for qc0 in range(0, NQ, 4):
                nq = min(4, NQ - qc0)
                at = ps_av.tile([128, 4, BCOLS], F32, tag="av")
                for qi in range(nq):
                    qc = qc0 + qi
                    for kc in range(NQ):
                        nc.tensor.matmul(
                            at[:, qi, :],
                            lhsT=et[:, kc, qc * 128 : (qc + 1) * 128],
                            rhs=v_sb[:, kc, h, :],
                            start=(kc == 0),
                            stop=(kc == NQ - 1),
                        )
                for qi in range(nq):
                    qc = qc0 + qi
                    rc = smalls.tile([128, 1], F32, tag="rc")
                    nc.vector.reciprocal(rc, at[:, qi, HD : HD + 1])
                    nc.vector.tensor_scalar_mul(
                        out=attn_sb[:, qc, h * HD : (h + 1) * HD],
                        in0=at[:, qi, 0:HD],
                        scalar1=rc,
                    )

        def emit_transposes(dc):
            # transpose the [*, dc*128:(dc+1)*128] slab of attn_out;
            # only needs heads 2*dc and 2*dc+1
            for qc in range(NQ):
                pt = ps_mm.tile([128, 128], BF16, tag="mm")
                nc.tensor.transpose(
                    pt[:], attn_sb[:, qc, dc * 128 : (dc + 1) * 128], ident[:]
                )
                nc.vector.tensor_copy(
                    out=attnT_sb[:, dc, qc * 128 : (qc + 1) * 128], in_=pt[:]
                )

        # ---- interleaved emission: pair p unlocks heads 2p, 2p+1 ----
        npairs = NP
        for p in range(npairs):
            emit_qkT(p)        # q chunk for heads 2p, 2p+1
            emit_qkT(NP + p)   # k chunk for heads 2p, 2p+1
            emit_scores(2 * p)
            if 2 * p + 1 < H:
                emit_scores(2 * p + 1)
            if p == 0:
                emit_v()       # PE fills v while ACT exps heads 0-1
            else:
                emit_av(2 * (p - 1))
                emit_av(2 * (p - 1) + 1)
                emit_transposes(p - 1)
        emit_av(2 * (npairs - 1))
        emit_av(2 * (npairs - 1) + 1)
        emit_transposes(npairs - 1)

        # ---- proj: out = attn_out @ w_proj + b ----
        out_sb = persist.tile([128, NQ, D], F32, tag="out")
        for qc in range(NQ):
            for s, L in pwins:
                pt = ps_mm.tile([128, 512], F32, tag="mm")
                for dc in range(DP):
                    nc.tensor.matmul(
                        pt[:, :L],
                        lhsT=attnT_sb[:, dc, qc * 128 : (qc + 1) * 128],
                        rhs=wp_sb[:, dc, s : s + L],
                        start=(dc == 0),
                        stop=(dc == DP - 1),
                    )
                nc.vector.tensor_add(
                    out_sb[:, qc, s : s + L], pt[:, :L], bp_sb[:, s : s + L]
                )
            nc.sync.dma_start(out=out_v[:, qc, :], in_=out_sb[:, qc, :])

    nc.compile()
    return nc


def _get_nc(n=N):
    if n not in _CACHED_NC:
        _CACHED_NC[n] = build_graph(n)
    return _CACHED_NC[n]


def make_in_maps(x, w_qkv, b_qkv, w_proj, b_proj):
    bf = ml_dtypes.bfloat16
    wq = np.ascontiguousarray(np.asarray(w_qkv, dtype=np.float32).astype(bf))
    wp = np.ascontiguousarray(np.asarray(w_proj, dtype=np.float32).astype(bf))
    bq = np.ascontiguousarray(np.asarray(b_qkv, dtype=np.float32))
    bp = np.ascontiguousarray(np.asarray(b_proj, dtype=np.float32))
    in_maps = []
    for b in range(x.shape[0]):
        xt = np.ascontiguousarray(np.asarray(x[b], dtype=np.float32).astype(bf).T)
        in_maps.append(
            {"xt": xt, "w_qkv": wq, "b_qkv": bq, "w_proj": wp, "b_proj": bp}
        )
    return in_maps


def run(x, w_qkv, b_qkv, w_proj, b_proj, **spmd_kwargs):
    nc = _get_nc(x.shape[1])
    in_maps = make_in_maps(x, w_qkv, b_qkv, w_proj, b_proj)
    res = run_bass_kernel_spmd(
        nc, in_maps, core_ids=list(range(len(in_maps))), **spmd_kwargs
    )
    outs = np.stack([np.asarray(r["out"], dtype=np.float32) for r in res.results])
    return outs, res


def kernel(x, w_qkv, b_qkv, w_proj, b_proj):
    outs, _ = run(x, w_qkv, b_qkv, w_proj, b_proj)
    return outs


# revision 11
# speedup vs baseline: 3.7595x; 3.7595x over previous
"""Multi-head attention forward on 8 TRN2 NeuronCores.

Data-parallel over batch: B=8 batch elements -> one per core, zero
collectives. Per core (one batch element, N=1024 tokens, D=768, H=12
heads of HD=64):

  1. qkT = (x @ w_qkv[:, :2D]).T + b      transposed layout [2D, N]
  2. v   =  x @ w_qkv[:, 2D:]  + b        natural layout [N, D], stored
                                          per-head with a ones column
  3. per head: scoresT[k, q] = k_h q_h^T  (K=64 matmuls)
     expT = exp(SCALE * scoresT)          bf16, no max subtraction
                                          (scores ~ N(0,1), exp safe)
  4. av[q, 0:64] = sum_k expT * v_h ; av[q, 64] = sum_k expT  (ones col
     gives the softmax denominator for free); normalize per-partition
  5. attn_out -> PE transpose -> proj matmul + bias -> out

Everything computes in bf16 with fp32 PSUM accumulation (fp8 was
measured at 3.5-5% output error on every path -- the 2e-2 gate rules
it out; the weighted-average structure of attention does not attenuate
v/p quantization noise because the output is itself noise-sized).

Emission is interleaved per head-pair so ACT (the exp engine, ~110us
total) starts early and overlaps the whole PE timeline.

Host-side prep (outside the HW kernel): shard x over batch, transpose
x to [D, N], cast x/w to bf16.
"""

from contextlib import ExitStack

import numpy as np
import ml_dtypes

import concourse.bass as bass
import concourse.mybir as mybir
import concourse.tile as tile
from concourse import bacc
from concourse.masks import make_identity
from concourse.bass_utils import run_bass_kernel_spmd

B, N, D, H = 8, 1024, 768, 12
HD = D // H  # 64
SCALE = HD ** -0.5
BF16 = mybir.dt.bfloat16
F32 = mybir.dt.float32
BCOLS = HD + 1  # v block width per head: 64 v columns + ones column

_CACHED_NC = {}


def build_graph(n=N):
    """Build the single-core Bass graph (SPMD: same graph on all 8 cores)."""
    assert n % 128 == 0
    DP = D // 128           # 6 chunks of the model dim
    NQ = n // 128           # seq chunks
    NP = 2 * D // 128 // 2  # 6 head-pair chunks (q or k each)
    qwins = [(s, min(512, n - s)) for s in range(0, n, 512)]
    pwins = [(s, min(384, D - s)) for s in range(0, D, 384)]

    nc = bacc.Bacc()
    xt = nc.declare_dram_parameter("xt", [D, n], BF16, isOutput=False)
    wqkv = nc.declare_dram_parameter("w_qkv", [D, 3 * D], BF16, isOutput=False)
    bqkv = nc.declare_dram_parameter("b_qkv", [3 * D], F32, isOutput=False)
    wproj = nc.declare_dram_parameter("w_proj", [D, D], BF16, isOutput=False)
    bproj = nc.declare_dram_parameter("b_proj", [D], F32, isOutput=False)
    out = nc.declare_dram_parameter("out", [n, D], F32, isOutput=True)

    xt_v = xt[:].rearrange("(dc p) n -> p dc n", p=128)
    wqkv_v = wqkv[:].rearrange("(dc p) m -> p dc m", p=128)
    wproj_v = wproj[:].rearrange("(dc p) m -> p dc m", p=128)
    out_v = out[:].rearrange("(c p) m -> p c m", p=128)
    bqk_v = bqkv[: 2 * D].rearrange("(j p) -> p j", p=128)

    def bcast(ap, p=128):
        return bass.AP(tensor=ap.tensor, offset=ap.offset, ap=[[0, p]] + list(ap.ap))

    with ExitStack() as ctx:
        tc = ctx.enter_context(tile.TileContext(nc))
        const = ctx.enter_context(tc.tile_pool(name="const", bufs=1))
        persist = ctx.enter_context(tc.tile_pool(name="persist", bufs=1))
        exps = ctx.enter_context(tc.tile_pool(name="exps", bufs=2))
        smalls = ctx.enter_context(tc.tile_pool(name="smalls", bufs=4))
        ps_mm = ctx.enter_context(tc.tile_pool(name="ps_mm", bufs=2, space="PSUM"))
        ps_sc = ctx.enter_context(tc.tile_pool(name="ps_sc", bufs=2, space="PSUM"))
        ps_av = ctx.enter_context(tc.tile_pool(name="ps_av", bufs=2, space="PSUM"))

        # ---- loads ----
        xt_sb = persist.tile([128, DP, n], BF16, tag="xt")
        for dc in range(DP):
            nc.sync.dma_start(out=xt_sb[:, dc, :], in_=xt_v[:, dc, :])
        wq_sb = persist.tile([128, DP, 3 * D], BF16, tag="wq")
        for dc in range(DP):
            nc.sync.dma_start(out=wq_sb[:, dc, :], in_=wqkv_v[:, dc, :])
        # biases routed through one DVE copy so later DVE adds depend on
        # them via same-engine program order (walrus allows 1 wait/inst)
        bqk_ld = const.tile([128, 2 * D // 128], F32, tag="bqk_ld")
        nc.gpsimd.dma_start(out=bqk_ld[:], in_=bqk_v)
        bqk_sb = const.tile([128, 2 * D // 128], F32, tag="bqk")
        nc.vector.tensor_copy(out=bqk_sb[:], in_=bqk_ld[:])
        bv_ld = const.tile([128, D], F32, tag="bv_ld")
        nc.gpsimd.dma_start(out=bv_ld[:], in_=bcast(bqkv[2 * D : 3 * D]))
        bv_sb = const.tile([128, D], F32, tag="bv")
        nc.vector.tensor_copy(out=bv_sb[:], in_=bv_ld[:])
        bp_ld = const.tile([128, D], F32, tag="bp_ld")
        nc.gpsimd.dma_start(out=bp_ld[:], in_=bcast(bproj[:]))
        bp_sb = const.tile([128, D], F32, tag="bp")
        nc.vector.tensor_copy(out=bp_sb[:], in_=bp_ld[:])
        wp_sb = persist.tile([128, DP, D], BF16, tag="wp")
        for dc in range(DP):
            nc.sync.dma_start(out=wp_sb[:, dc, :], in_=wproj_v[:, dc, :])
        ident = const.tile([128, 128], BF16, tag="ident")
        make_identity(nc, ident)

        qkT_sb = persist.tile([128, 2 * D // 128, n], BF16, tag="qkT")
        v_sb = persist.tile([128, NQ, H, BCOLS], BF16, tag="v")
        nc.vector.memset(v_sb[:, :, :, HD : HD + 1], 1.0)
        attn_sb = persist.tile([128, NQ, D], BF16, tag="attn")
        attnT_sb = persist.tile([128, DP, n], BF16, tag="attnT")
        et_tiles = [None] * H

        def emit_qkT(j):
            # one 128-wide chunk of the transposed q|k projection
            for s, L in qwins:
                pt = ps_mm.tile([128, 512], F32, tag="mm")
                for dc in range(DP):
                    nc.tensor.matmul(
                        pt[:, :L],
                        lhsT=wq_sb[:, dc, j * 128 : (j + 1) * 128],
                        rhs=xt_sb[:, dc, s : s + L],
                        start=(dc == 0),
                        stop=(dc == DP - 1),
                    )
                nc.vector.tensor_scalar_add(
                    out=qkT_sb[:, j, s : s + L],
                    in0=pt[:, :L],
                    scalar1=bqk_sb[:, j : j + 1],
                )

        def emit_v():
            for c in range(NQ):
                for s, L in pwins:
                    nh = L // HD
                    h0 = s // HD
                    pt = ps_mm.tile([128, 512], F32, tag="mm")
                    for dc in range(DP):
                        nc.tensor.matmul(
                            pt[:, :L],
                            lhsT=xt_sb[:, dc, c * 128 : (c + 1) * 128],
                            rhs=wq_sb[:, dc, 2 * D + s : 2 * D + s + L],
                            start=(dc == 0),
                            stop=(dc == DP - 1),
                        )
                    nc.vector.tensor_add(
                        v_sb[:, c, h0 : h0 + nh, 0:HD],
                        pt[:, :L].rearrange("p (h x) -> p h x", h=nh),
                        bv_sb[:, s : s + L].rearrange("p (h x) -> p h x", h=nh),
                    )

        def emit_scores(h):
            poff = (h % 2) * 64
            qch = h // 2
            kch = NP + h // 2
            et = exps.tile([128, NQ, n], BF16, tag="exp")
            et_tiles[h] = et
            for kc in range(NQ):
                st = ps_sc.tile([128, n], F32, tag="sc")
                for s, L in qwins:
                    nc.tensor.matmul(
                        st[:, s : s + L],
                        lhsT=qkT_sb[poff : poff + 64, kch, kc * 128 : (kc + 1) * 128],
                        rhs=qkT_sb[poff : poff + 64, qch, s : s + L],
                        start=True,
                        stop=True,
                    )
                nc.scalar.activation(
                    out=et[:, kc, :],
                    in_=st[:, :],
                    func=mybir.ActivationFunctionType.Exp,
                    scale=SCALE,
                )

        def emit_av(h):
            et = et_tiles[h]
            for qc0 in range(0, NQ, 4):
                nq = min(4, NQ - qc0)
                at = ps_av.tile([128, 4, BCOLS], F32, tag="av")
                for qi in range(nq):
                    qc = qc0 + qi
                    for kc in range(NQ):
                        nc.tensor.matmul(
                            at[:, qi, :],
                            lhsT=et[:, kc, qc * 128 : (qc + 1) * 128],
                            rhs=v_sb[:, kc, h, :],
                            start=(kc == 0),
                            stop=(kc == NQ - 1),
                        )
                for qi in range(nq):
                    qc = qc0 + qi
                    rc = smalls.tile([128, 1], F32, tag="rc")
                    nc.vector.reciprocal(rc, at[:, qi, HD : HD + 1])
                    nc.vector.tensor_scalar_mul(
                        out=attn_sb[:, qc, h * HD : (h + 1) * HD],
                        in0=at[:, qi, 0:HD],
                        scalar1=rc,
                    )

        def emit_transposes(dc):
            # transpose the [*, dc*128:(dc+1)*128] slab of attn_out;
            # only needs heads 2*dc and 2*dc+1
            for qc in range(NQ):
                pt = ps_mm.tile([128, 128], BF16, tag="mm")
                nc.tensor.transpose(
                    pt[:], attn_sb[:, qc, dc * 128 : (dc + 1) * 128], ident[:]
                )
                nc.vector.tensor_copy(
                    out=attnT_sb[:, dc, qc * 128 : (qc + 1) * 128], in_=pt[:]
                )

        out_sb = persist.tile([128, NQ, D], F32, tag="out")

        def emit_proj_partial(dc):
            # one K=128 slab of the output projection, accumulated into
            # out_sb on DVE -- spreads proj across the attention phase
            # instead of a 15us PE tail gated on the last head
            for qc in range(NQ):
                for s, L in pwins:
                    pt = ps_mm.tile([128, 512], F32, tag="mm")
                    nc.tensor.matmul(
                        pt[:, :L],
                        lhsT=attnT_sb[:, dc, qc * 128 : (qc + 1) * 128],
                        rhs=wp_sb[:, dc, s : s + L],
                        start=True,
                        stop=True,
                    )
                    if dc == 0:
                        nc.vector.tensor_add(
                            out_sb[:, qc, s : s + L], pt[:, :L], bp_sb[:, s : s + L]
                        )
                    else:
                        nc.vector.tensor_add(
                            out_sb[:, qc, s : s + L],
                            out_sb[:, qc, s : s + L],
                            pt[:, :L],
                        )
                if dc == DP - 1:
                    nc.sync.dma_start(out=out_v[:, qc, :], in_=out_sb[:, qc, :])

        # ---- interleaved emission: pair p unlocks heads 2p, 2p+1 ----
        npairs = NP
        for p in range(npairs):
            emit_qkT(p)        # q chunk for heads 2p, 2p+1
            emit_qkT(NP + p)   # k chunk for heads 2p, 2p+1
            emit_scores(2 * p)
            if 2 * p + 1 < H:
                emit_scores(2 * p + 1)
            if p == 0:
                emit_v()       # PE fills v while ACT exps heads 0-1
            else:
                emit_av(2 * (p - 1))
                emit_av(2 * (p - 1) + 1)
                emit_transposes(p - 1)
                emit_proj_partial(p - 1)
        emit_av(2 * (npairs - 1))
        emit_av(2 * (npairs - 1) + 1)
        emit_transposes(npairs - 1)
        emit_proj_partial(npairs - 1)

    nc.compile()
    return nc


def _get_nc(n=N):
    if n not in _CACHED_NC:
        _CACHED_NC[n] = build_graph(n)
    return _CACHED_NC[n]


def make_in_maps(x, w_qkv, b_qkv, w_proj, b_proj):
    bf = ml_dtypes.bfloat16
    wq = np.ascontiguousarray(np.asarray(w_qkv, dtype=np.float32).astype(bf))
    wp = np.ascontiguousarray(np.asarray(w_proj, dtype=np.float32).astype(bf))
    bq = np.ascontiguousarray(np.asarray(b_qkv, dtype=np.float32))
    bp = np.ascontiguousarray(np.asarray(b_proj, dtype=np.float32))
    in_maps = []
    for b in range(x.shape[0]):
        xt = np.ascontiguousarray(np.asarray(x[b], dtype=np.float32).astype(bf).T)
        in_maps.append(
            {"xt": xt, "w_qkv": wq, "b_qkv": bq, "w_proj": wp, "b_proj": bp}
        )
    return in_maps


def run(x, w_qkv, b_qkv, w_proj, b_proj, **spmd_kwargs):
    nc = _get_nc(x.shape[1])
    in_maps = make_in_maps(x, w_qkv, b_qkv, w_proj, b_proj)
    res = run_bass_kernel_spmd(
        nc, in_maps, core_ids=list(range(len(in_maps))), **spmd_kwargs
    )
    outs = np.stack([np.asarray(r["out"], dtype=np.float32) for r in res.results])
    return outs, res


def kernel(x, w_qkv, b_qkv, w_proj, b_proj):
    outs, _ = run(x, w_qkv, b_qkv, w_proj, b_proj)
    return outs
